# revision 1
# baseline (speedup 1.0000x reference)
"""Graphormer multi-head attention on 8 Trainium2 NeuronCores.

Problem (hardcoded): B=16, N=512, HIDDEN=768, 12 heads x 64.
  q = x @ Wq.T ; k = x @ Wk.T ; v = x @ Wv.T + bv
  scores = (q.k / sqrt(768)) + (spatial + edge)[:, None]
  out = softmax(scores) @ v ; y = out @ Wo.T + bo

Sharding: data-parallel over batch, 2 batches per core on 8 cores.

Per-core kernel strategy (all matmuls bf16 with fp32 PSUM accumulate):
  - Weights transposed once via PE transpose-mode, cast bf16 on copyback;
    softmax scale folded into WqT.  x transposed per batch the same way.
  - qT, kT computed transposed [hidden', nq]; v computed natural [nq, hidden']
    (bv folded in as a K=1 rank-1 matmul with a ones row).
  - Attention in the S^T layout: S.T[nk, nq] = kT.T @ qT per head (K=64
    matmuls; even/odd heads sit in partition halves so the PE row-tiles them
    concurrently).  Two nk-chunks share one 2-bank PSUM tile so exp runs as
    [128,1024] ScalarE ops.  Bias enters as exp(S) = exp(qk) * exp(bias) with
    E.T = exp((spatial+edge).T) shared across heads (VectorE bf16 multiply).
  - PV: O.T[d, nq] = V.T @ expS.T with V natural-layout as lhsT; an all-ones
    lhsT writes replicated row-sums into the other 64 PSUM partitions
    (column-tiled, concurrent), so normalization is one reciprocal + one
    multiply per head pair, partition-aligned.
  - y = O @ Wo.T + bo with lhsT = O.T (no output transpose), bo as rank-1.
"""

import numpy as np

B, N, H = 16, 512, 768
NH, HD = 12, 64
NCORES = 8
PB = B // NCORES  # batches per core
P = 128
KC = H // P   # 6 k-chunks of hidden
NQC = N // P  # 4 chunks of sequence
SCALE = float(H) ** -0.5

_COMPILED = None


def _build():
    import concourse.bacc as bacc
    import concourse.tile as tile
    import concourse.mybir as mybir
    from concourse.masks import make_identity

    f32 = mybir.dt.float32
    bf16 = mybir.dt.bfloat16
    Exp = mybir.ActivationFunctionType.Exp

    nc = bacc.Bacc("TRN2", target_bir_lowering=False, debug=False,
                   enable_asserts=False, num_devices=NCORES)

    x_d = nc.dram_tensor("x", [PB, N, H], f32, kind="ExternalInput").ap()
    sp_d = nc.dram_tensor("spatial", [PB, N, N], f32, kind="ExternalInput").ap()
    ed_d = nc.dram_tensor("edge", [PB, N, N], f32, kind="ExternalInput").ap()
    wq_d = nc.dram_tensor("Wq", [H, H], f32, kind="ExternalInput").ap()
    wk_d = nc.dram_tensor("Wk", [H, H], f32, kind="ExternalInput").ap()
    wv_d = nc.dram_tensor("Wv", [H, H], f32, kind="ExternalInput").ap()
    bv_d = nc.dram_tensor("bv", [H], f32, kind="ExternalInput").ap()
    wo_d = nc.dram_tensor("Wo", [H, H], f32, kind="ExternalInput").ap()
    bo_d = nc.dram_tensor("bo", [H], f32, kind="ExternalInput").ap()
    y_d = nc.dram_tensor("y", [PB, N, H], f32, kind="ExternalOutput").ap()

    with tile.TileContext(nc) as tc:
        with (
            tc.tile_pool(name="consts", bufs=1) as consts,
            tc.tile_pool(name="weights", bufs=1) as weights,
            tc.tile_pool(name="stage", bufs=2) as stage,
            tc.tile_pool(name="acts", bufs=2) as acts,
            tc.tile_pool(name="soft", bufs=6) as soft,
            tc.tile_pool(name="outs", bufs=2) as outs,
            tc.tile_pool(name="psum", bufs=3, space="PSUM") as psum,
        ):
            ident = consts.tile([P, P], f32)
            make_identity(nc, ident)
            ones_r1 = consts.tile([1, P], bf16)
            nc.vector.memset(ones_r1, 1.0)
            ones_sum = consts.tile([P, HD], bf16)
            nc.vector.memset(ones_sum, 1.0)

            # biases, cast to bf16 rows for rank-1 matmuls
            bv_f = consts.tile([1, H], f32)
            nc.sync.dma_start(out=bv_f, in_=bv_d[None, :])
            bv_sb = consts.tile([1, H], bf16)
            nc.vector.tensor_copy(bv_sb, bv_f)
            bo_f = consts.tile([1, H], f32)
            nc.sync.dma_start(out=bo_f, in_=bo_d[None, :])
            bo_sb = consts.tile([1, H], bf16)
            nc.vector.tensor_copy(bo_sb, bo_f)

            def transpose_block(dst_sb, src_blocks, copy_engine, scl=1.0):
                """PE-transpose a row of [128,128] f32 blocks into PSUM, then
                cast-copy into dst_sb (bf16)."""
                nblk = len(src_blocks)
                pa = psum.tile([P, nblk * P], f32,
                               tag=("ps" if nblk == 4 else "ps256"),
                               bufs=(2 if nblk == 4 else 2), name="pa")
                for i, blk in enumerate(src_blocks):
                    nc.tensor.transpose(pa[:, i * P:(i + 1) * P], blk, ident)
                if copy_engine == "act":
                    nc.scalar.mul(dst_sb, pa, scl)
                else:
                    nc.vector.tensor_scalar_mul(dst_sb, pa, scl)

            # ---- weights: load [m,k] f32, PE-transpose to [k,m], cast bf16 ----
            wt = {}
            for name, wd, scl in (("wq", wq_d, SCALE), ("wk", wk_d, 1.0),
                                  ("wv", wv_d, 1.0), ("wo", wo_d, 1.0)):
                wsb = stage.tile([P, KC, H], f32, tag="hstage")
                nc.sync.dma_start(
                    out=wsb, in_=wd.rearrange("(c p) k -> p c k", p=P))
                wtile = weights.tile([P, KC, H], bf16, name=f"{name}T")
                for kc in range(KC):
                    ksl = slice(kc * P, (kc + 1) * P)
                    transpose_block(wtile[:, kc, 0:N],
                                    [wsb[:, mi, ksl] for mi in range(4)],
                                    "act", scl)
                    transpose_block(wtile[:, kc, N:H],
                                    [wsb[:, mi, ksl] for mi in range(4, 6)],
                                    "act", scl)
                wt[name] = wtile

            for b in range(PB):
                # ---- xT: load x, PE-transpose, cast bf16 ----
                xsb = stage.tile([P, NQC, H], f32, tag="hstage")
                nc.sync.dma_start(
                    out=xsb, in_=x_d[b].rearrange("(c p) k -> p c k", p=P))
                xT = acts.tile([P, KC, N], bf16, tag="xT")
                for kc in range(KC):
                    ksl = slice(kc * P, (kc + 1) * P)
                    transpose_block(xT[:, kc, :],
                                    [xsb[:, ni, ksl] for ni in range(NQC)],
                                    "dve")

                # ---- E.T = exp((spatial+edge).T) bf16 ----
                ET = acts.tile([P, NQC, N], bf16, tag="ET")
                bsum = []
                for ni in range(NQC):
                    ssb = stage.tile([P, N], f32, tag="bias_s")
                    nc.sync.dma_start(out=ssb, in_=sp_d[b, ni * P:(ni + 1) * P, :])
                    esb = stage.tile([P, N], f32, tag="bias_e")
                    nc.sync.dma_start(out=esb, in_=ed_d[b, ni * P:(ni + 1) * P, :])
                    bs = stage.tile([P, N], f32, tag="bias_sum", bufs=4)
                    nc.vector.tensor_add(bs, ssb, esb)
                    bsum.append(bs)
                for ki in range(NQC):
                    pb_ = psum.tile([P, N], f32, tag="ps", bufs=2, name="pb_")
                    for ni in range(NQC):
                        nc.tensor.transpose(
                            pb_[:, ni * P:(ni + 1) * P],
                            bsum[ni][:, ki * P:(ki + 1) * P], ident)
                    nc.scalar.activation(ET[:, ki, :], pb_, Exp)

                # ---- projections ----
                qT = acts.tile([P, KC, N], bf16, tag="qT")
                kT = acts.tile([P, KC, N], bf16, tag="kT")
                for dst, wname in ((qT, "wq"), (kT, "wk")):
                    for mi in range(KC):
                        pq = psum.tile([P, N], f32, tag="ps", bufs=2)
                        for kc in range(KC):
                            nc.tensor.matmul(
                                pq, wt[wname][:, kc, mi * P:(mi + 1) * P],
                                xT[:, kc, :], start=(kc == 0), stop=(kc == KC - 1))
                        nc.scalar.copy(dst[:, mi, :], pq)

                vsb = acts.tile([P, NQC, H], bf16, tag="v")
                for ni in range(NQC):
                    pva = psum.tile([P, N], f32, tag="ps", bufs=2)
                    pvb = psum.tile([P, H - N], f32, tag="ps256", bufs=2)
                    for kc in range(KC):
                        lhs = xT[:, kc, ni * P:(ni + 1) * P]
                        nc.tensor.matmul(pva, lhs, wt["wv"][:, kc, 0:N],
                                         start=(kc == 0), stop=False)
                        nc.tensor.matmul(pvb, lhs, wt["wv"][:, kc, N:H],
                                         start=(kc == 0), stop=False)
                    nc.tensor.matmul(pva, ones_r1, bv_sb[:, 0:N],
                                     start=False, stop=True)
                    nc.tensor.matmul(pvb, ones_r1, bv_sb[:, N:H],
                                     start=False, stop=True)
                    nc.vector.tensor_scalar_mul(vsb[:, ni, 0:N], pva, 1.0)
                    nc.vector.tensor_scalar_mul(vsb[:, ni, N:H], pvb, 1.0)

                # ---- attention, head pairs ----
                ET_flat = ET.rearrange("p c n -> p (c n)")
                OT = outs.tile([P, KC, N], bf16, tag="OT")
                for pr in range(NH // 2):
                    # expst[hi][half] covers nk chunks (2*half, 2*half+1)
                    expst = [[None, None], [None, None]]
                    for hi, ro in ((0, 0), (1, HD)):
                        rs = slice(ro, ro + HD)
                        for half in range(2):
                            pqk = psum.tile([P, 2 * N], f32, tag="ps2", bufs=2)
                            for j in range(2):
                                ki = 2 * half + j
                                nc.tensor.matmul(
                                    pqk[:, j * N:(j + 1) * N],
                                    kT[rs, pr, ki * P:(ki + 1) * P],
                                    qT[rs, pr, :],
                                    start=True, stop=True)
                            es = soft.tile([P, 2 * N], bf16, tag="expst")
                            nc.scalar.activation(es, pqk, Exp)
                            nc.vector.tensor_mul(
                                es, es,
                                ET_flat[:, half * 2 * N:(half + 1) * 2 * N])
                            expst[hi][half] = es
                    po = psum.tile([P, N], f32, tag="ps", bufs=2)
                    prs = psum.tile([P, N], f32, tag="ps", bufs=2)
                    h0, h1 = 2 * pr, 2 * pr + 1
                    for ki in range(NQC):
                        st, sp_ = (ki == 0), (ki == NQC - 1)
                        e0 = expst[0][ki // 2][:, (ki % 2) * N:(ki % 2 + 1) * N]
                        e1 = expst[1][ki // 2][:, (ki % 2) * N:(ki % 2 + 1) * N]
                        nc.tensor.matmul(
                            po[0:HD, :], vsb[:, ki, h0 * HD:(h0 + 1) * HD],
                            e0, start=st, stop=sp_, skip_group_check=True)
                        nc.tensor.matmul(
                            po[HD:P, :], vsb[:, ki, h1 * HD:(h1 + 1) * HD],
                            e1, start=st, stop=sp_, skip_group_check=True)
                        nc.tensor.matmul(
                            prs[0:HD, :], ones_sum, e0,
                            start=st, stop=sp_, skip_group_check=True)
                        nc.tensor.matmul(
                            prs[HD:P, :], ones_sum, e1,
                            start=st, stop=sp_, skip_group_check=True)
                    rr = soft.tile([P, N], f32, tag="rsrecip", bufs=2)
                    nc.vector.reciprocal_approx_fast(rr, prs)
                    nc.vector.tensor_mul(OT[:, pr, :], po, rr)

                # ---- y = O @ Wo.T + bo ----
                for ni in range(NQC):
                    pya = psum.tile([P, N], f32, tag="ps", bufs=2)
                    pyb = psum.tile([P, H - N], f32, tag="ps256", bufs=2)
                    for jc in range(KC):
                        lhs = OT[:, jc, ni * P:(ni + 1) * P]
                        nc.tensor.matmul(pya, lhs, wt["wo"][:, jc, 0:N],
                                         start=(jc == 0), stop=False)
                        nc.tensor.matmul(pyb, lhs, wt["wo"][:, jc, N:H],
                                         start=(jc == 0), stop=False)
                    nc.tensor.matmul(pya, ones_r1, bo_sb[:, 0:N],
                                     start=False, stop=True)
                    nc.tensor.matmul(pyb, ones_r1, bo_sb[:, N:H],
                                     start=False, stop=True)
                    ysb = outs.tile([P, H], f32, tag="ysb", bufs=3)
                    nc.vector.tensor_scalar_mul(ysb[:, 0:N], pya, 1.0)
                    nc.vector.tensor_scalar_mul(ysb[:, N:H], pyb, 1.0)
                    nc.sync.dma_start(
                        out=y_d[b, ni * P:(ni + 1) * P, :], in_=ysb)

    nc.compile()
    return nc


def kernel(x, spatial_encoding, edge_encoding, Wq, Wk, Wv, bv, Wo, bo):
    global _COMPILED
    from concourse.bass_utils import run_bass_kernel_spmd

    if _COMPILED is None:
        _COMPILED = _build()
    nc = _COMPILED

    x = np.asarray(x, dtype=np.float32)
    sp = np.asarray(spatial_encoding, dtype=np.float32)
    ed = np.asarray(edge_encoding, dtype=np.float32)
    shared = {
        "Wq": np.asarray(Wq, np.float32), "Wk": np.asarray(Wk, np.float32),
        "Wv": np.asarray(Wv, np.float32), "bv": np.asarray(bv, np.float32),
        "Wo": np.asarray(Wo, np.float32), "bo": np.asarray(bo, np.float32),
    }
    in_maps = []
    for c in range(NCORES):
        sl = slice(c * PB, (c + 1) * PB)
        in_maps.append({"x": x[sl], "spatial": sp[sl], "edge": ed[sl], **shared})

    res = run_bass_kernel_spmd(nc, in_maps, list(range(NCORES)))
    return np.concatenate([res.results[c]["y"] for c in range(NCORES)], axis=0)



# revision 18
# speedup vs baseline: 1.2653x; 1.2653x over previous
"""Graphormer multi-head attention on 8 Trainium2 NeuronCores.

Problem (hardcoded): B=16, N=512, HIDDEN=768, 12 heads x 64.
  q = x @ Wq.T ; k = x @ Wk.T ; v = x @ Wv.T + bv
  scores = (q.k / sqrt(768)) + (spatial + edge)[:, None]
  out = softmax(scores) @ v ; y = out @ Wo.T + bo
Sharding: data-parallel over batch, 2 batches per core on 8 cores.

Per-core kernel strategy:
  - All layout work happens on the HOST: x/spatial/edge are pre-transposed
    and packed; weights are pre-transposed and cast to bf16.  bv is
    folded into bo' = bo + Wo@bv on the host (valid because softmax rows
    sum to 1).  No PE transposes.
  - All matmuls bf16 with fp32 PSUM accumulation; the 1/sqrt(768)
    softmax scale is folded into Wq on the host.
  - Attention in the S^T layout: S.T[nk, nq] = kT.T @ qT per head.
    exp(S^T * s) on ScalarE -> bf16, then one DVE multiply with
    E^T = exp((spatial+edge)^T) shared across heads.
  - PV per head: lhsT = [v_head | ones] (ones live in dedicated slots of
    the v tile), so one matmul chain yields both O^T (64 rows) and the
    softmax denominator replicated across the other 64 rows -- no
    separate row-sum matmuls.  A tiny stride-0-partition DMA broadcasts
    the denominator row into SBUF on the partitions where O^T lives,
    keeping the reciprocal+normalize DVE ops partition-aligned.
  - y = O @ Wo.T + bo' with lhsT = O^T; bo' enters as a K=1 matmul and
    the result is DMA'd to DRAM straight out of PSUM.
  - Emission interleaves batch b+1's projections into batch b's
    attention loop so the PE never starves.
"""

import numpy as np
import ml_dtypes

B, N, H = 16, 512, 768
NH, HD = 12, 64
NCORES = 8
PB = B // NCORES  # batches per core
P = 128
KC = H // P   # 6 hidden chunks of 128
NQC = N // P  # 4 sequence chunks of 128
SCALE = float(H) ** -0.5        # folded into Wq on the host

BFNP = ml_dtypes.bfloat16

_COMPILED = None

# debug toggles (set before _build for HW-vs-sim bisection)
DBG_NO_PBCAST = False    # replace partition_broadcast with memset(1.0)
DBG_NO_ACCUM = False     # replace DMA-accum bias sum with DVE add
DBG_NO_GMEMSET = False   # vsb ones via DVE memset instead of gpsimd
DBG_SEQUENTIAL = False   # no cross-batch interleaving in emission order
DBG_DUMP = False         # DMA b1 intermediates to DRAM debug outputs


def _build():
    import concourse.bacc as bacc
    import concourse.tile as tile
    import concourse.mybir as mybir

    f32 = mybir.dt.float32
    bf16 = mybir.dt.bfloat16
    Exp = mybir.ActivationFunctionType.Exp
    ADD = mybir.AluOpType.add

    nc = bacc.Bacc("TRN2", target_bir_lowering=False, debug=False,
                   enable_asserts=False, num_devices=NCORES)

    xb_d = nc.dram_tensor("xb", [PB, P, KC, N], bf16, kind="ExternalInput").ap()
    sp_d = nc.dram_tensor("spT", [PB, P, NQC, N], f32, kind="ExternalInput").ap()
    ed_d = nc.dram_tensor("edT", [PB, P, NQC, N], f32, kind="ExternalInput").ap()
    wq_d = nc.dram_tensor("wqT", [P, KC, H], bf16, kind="ExternalInput").ap()
    wk_d = nc.dram_tensor("wkT", [P, KC, H], bf16, kind="ExternalInput").ap()
    wv_d = nc.dram_tensor("wvT", [P, KC, H], bf16, kind="ExternalInput").ap()
    wo_d = nc.dram_tensor("woT", [P, KC, H], bf16, kind="ExternalInput").ap()
    bo_d = nc.dram_tensor("bo2", [H], bf16, kind="ExternalInput").ap()
    y_d = nc.dram_tensor("y", [PB, N, H], f32, kind="ExternalOutput").ap()
    if DBG_DUMP:
        dbg_qT = nc.dram_tensor("dbg_qT", [P, KC, N], bf16, kind="ExternalOutput").ap()
        dbg_kT = nc.dram_tensor("dbg_kT", [P, KC, N], bf16, kind="ExternalOutput").ap()
        dbg_ET = nc.dram_tensor("dbg_ET", [P, NQC, N], bf16, kind="ExternalOutput").ap()
        dbg_vsb = nc.dram_tensor("dbg_vsb", [P, NQC, NH, P], bf16, kind="ExternalOutput").ap()
        dbg_OT = nc.dram_tensor("dbg_OT", [P, KC, N], bf16, kind="ExternalOutput").ap()

    with tile.TileContext(nc) as tc:
        with (
            tc.tile_pool(name="consts", bufs=1) as consts,
            tc.tile_pool(name="weights", bufs=1) as weights,
            tc.tile_pool(name="io", bufs=2) as io,
            tc.tile_pool(name="biasp", bufs=2) as biasp,
            tc.tile_pool(name="qk", bufs=2) as qkp,
            tc.tile_pool(name="vp", bufs=1) as vp,
            tc.tile_pool(name="soft", bufs=3) as soft,
            tc.tile_pool(name="zp", bufs=2) as zp,
            tc.tile_pool(name="op", bufs=2) as op_,
            tc.tile_pool(name="psum", bufs=2, space="PSUM") as psum,
        ):
            ones_r1 = consts.tile([1, P], bf16)
            nc.vector.memset(ones_r1, 1.0)
            bo2_sb = consts.tile([1, H], bf16)
            nc.sync.dma_start(out=bo2_sb, in_=bo_d[None, :])

            wqT = weights.tile([P, KC, H], bf16)
            nc.sync.dma_start(out=wqT, in_=wq_d)
            wkT = weights.tile([P, KC, H], bf16)
            nc.sync.dma_start(out=wkT, in_=wk_d)
            wvT = weights.tile([P, KC, H], bf16)
            nc.sync.dma_start(out=wvT, in_=wv_d)
            woT = weights.tile([P, KC, H], bf16)
            nc.sync.dma_start(out=woT, in_=wo_d)

            # v tiles: each head owns a contiguous 128-wide slot holding
            # [v_h | ones]; the ones columns make the PV matmul emit the
            # softmax denominator, replicated across PSUM partitions
            # 64:128, for free.
            vsb = [vp.tile([P, NQC, NH, P], bf16, name=f"vsb{i}")
                   for i in range(PB)]
            for t in vsb:
                nc.gpsimd.memset(t[:, :, :, HD:P], 1.0)

            # per-batch input tiles + bias path (DMA-accumulated sum, exp)
            xbt = []
            for b in range(PB):
                xb_t = io.tile([P, KC, N], bf16, tag="xb")
                nc.sync.dma_start(out=xb_t, in_=xb_d[b])
                xbt.append(xb_t)

            def emit_bias(b):
                bsum = biasp.tile([P, NQC, N], f32, tag="bsum", name="bsum")
                if DBG_NO_ACCUM:
                    esb = biasp.tile([P, NQC, N], f32, tag="esb", name="esb")
                    nc.sync.dma_start(out=bsum, in_=sp_d[b])
                    nc.sync.dma_start(out=esb, in_=ed_d[b])
                    nc.vector.tensor_add(bsum, bsum, esb)
                else:
                    nc.gpsimd.dma_start(out=bsum, in_=sp_d[b])
                    nc.gpsimd.dma_start(out=bsum, in_=ed_d[b], accum_op=ADD)
                ET = biasp.tile([P, NQC, N], bf16, tag="ET", name="ET")
                nc.scalar.activation(ET, bsum, Exp)
                return ET.rearrange("p c n -> p (c n)")

            qT, kT, OT = {}, {}, {}

            def emit_projqk(b, mi):
                for wt, dst in ((wqT, qT), (wkT, kT)):
                    pp = psum.tile([P, N], f32, tag="pp", name="pp")
                    for kc in range(KC):
                        nc.tensor.matmul(
                            pp, wt[:, kc, mi * P:(mi + 1) * P],
                            xbt[b][:, kc, :],
                            start=(kc == 0), stop=(kc == KC - 1))
                    nc.scalar.copy(dst[b][:, mi, :], pp)

            def emit_projv(b, ni):
                for half in range(2):
                    cols = slice(0, N) if half == 0 else slice(N, H)
                    width = cols.stop - cols.start
                    nh = width // HD       # heads covered by this psum tile
                    h0 = half * 8          # first head
                    pv = psum.tile([P, N], f32, tag="pp", name="pv")
                    for kc in range(KC):
                        nc.tensor.matmul(
                            pv[:, 0:width],
                            xbt[b][:, kc, ni * P:(ni + 1) * P],
                            wvT[:, kc, cols],
                            start=(kc == 0), stop=(kc == KC - 1))
                    pvh = pv[:, 0:width].rearrange("p (h d) -> p h d", d=HD)
                    nc.vector.tensor_copy(
                        vsb[b][:, ni, h0:h0 + nh, 0:HD], pvh)

            es_t = {}

            def emit_qk_head(b, h, ETflat):
                mi, r0 = h // 2, (h % 2) * HD
                rs = slice(r0, r0 + HD)
                es = soft.tile([P, 2 * N * 2], bf16, tag="es", name="es")
                for half in range(2):
                    pq = psum.tile([P, 2 * N], f32, tag="pqk", name="pq")
                    for j in range(2):
                        ki = 2 * half + j
                        nc.tensor.matmul(
                            pq[:, j * N:(j + 1) * N],
                            kT[b][rs, mi, ki * P:(ki + 1) * P],
                            qT[b][rs, mi, :],
                            start=True, stop=True)
                    nc.scalar.activation(
                        es[:, half * 2 * N:(half + 1) * 2 * N], pq, Exp)
                nc.vector.tensor_mul(es, es, ETflat)
                es_t[(b, h)] = es

            def emit_pv_head(b, h):
                es = es_t.pop((b, h))
                po = psum.tile([P, N], f32, tag="po", name="po")
                for ki in range(NQC):
                    # [v|ones] -> O rows 0:64, Z (replicated) rows 64:128
                    nc.tensor.matmul(po, vsb[b][:, ki, h, :],
                                     es[:, ki * N:(ki + 1) * N],
                                     start=(ki == 0), stop=(ki == NQC - 1))
                # move Z onto partitions 0:64 (cross-offset copy is the only
                # partition-crossing engine op that works on HW), reciprocal
                # at offset 0, then scale O into its OT slot (cross-offset
                # output is fine when both inputs share base partition 0).
                rr = zp.tile([P, N], f32, tag="rr", name="rr")
                if h % 2 == 0:
                    nc.scalar.copy(rr[0:HD, :], po[HD:P, :])
                else:
                    nc.vector.tensor_copy(rr[0:HD, :], po[HD:P, :])
                nc.vector.reciprocal_approx_fast(rr[0:HD, :], rr[0:HD, :])
                orow = slice((h % 2) * HD, (h % 2) * HD + HD)
                nc.vector.tensor_mul(OT[b][orow, h // 2, :], po[0:HD, :],
                                     rr[0:HD, :])

            def emit_y(b, ni):
                ysb = op_.tile([P, H], f32, tag="ysb", name="ysb", bufs=3)
                for half in range(2):
                    cols = slice(0, N) if half == 0 else slice(N, H)
                    width = cols.stop - cols.start
                    py = psum.tile([P, N], f32, tag="pp", name="py")
                    for jc in range(KC):
                        nc.tensor.matmul(
                            py[:, 0:width],
                            OT[b][:, jc, ni * P:(ni + 1) * P],
                            woT[:, jc, cols],
                            start=(jc == 0), stop=False)
                    nc.tensor.matmul(py[:, 0:width], ones_r1,
                                     bo2_sb[:, cols], start=False, stop=True)
                    if half == 0:
                        nc.scalar.copy(ysb[:, cols], py[:, 0:width])
                    else:
                        nc.vector.tensor_copy(ysb[:, cols], py[:, 0:width])
                nc.sync.dma_start(
                    out=y_d[b, ni * P:(ni + 1) * P, :], in_=ysb)

            # ---------------- emission schedule ----------------
            ET0 = emit_bias(0)
            for b in range(PB):
                qT[b] = qkp.tile([P, KC, N], bf16, tag="qT", name="qT")
                kT[b] = qkp.tile([P, KC, N], bf16, tag="kT", name="kT")
                OT[b] = op_.tile([P, KC, N], bf16, tag="OT", name="OT")

            ET1_holder = []
            if DBG_SEQUENTIAL:
                ET1 = emit_bias(1)
                ET1_holder.append(ET1)
                for b in range(PB):
                    for mi in range(KC):
                        emit_projqk(b, mi)
                    for ni in range(NQC):
                        emit_projv(b, ni)
                    for h in range(NH):
                        emit_qk_head(b, h, [ET0, ET1][b])
                        emit_pv_head(b, h)
                    for ni in range(NQC):
                        emit_y(b, ni)
            else:
                for mi in range(KC):
                    emit_projqk(0, mi)
                for ni in range(NQC):
                    emit_projv(0, ni)
                ET1 = emit_bias(1)
                ETs = [ET0, ET1]

                # attention(0) interleaved with projections(1)
                b1_chunks = ([("qk", mi) for mi in range(KC)]
                             + [("v", ni) for ni in range(NQC)])
                ci = 0
                for h in range(NH):
                    emit_qk_head(0, h, ETs[0])
                    if h >= 1:
                        emit_pv_head(0, h - 1)
                    if ci < len(b1_chunks):
                        kind, idx = b1_chunks[ci]
                        ci += 1
                        (emit_projqk if kind == "qk" else emit_projv)(1, idx)
                while ci < len(b1_chunks):
                    kind, idx = b1_chunks[ci]
                    ci += 1
                    (emit_projqk if kind == "qk" else emit_projv)(1, idx)
                emit_pv_head(0, NH - 1)

                # attention(1) interleaved with output proj(0)
                for h in range(NH):
                    emit_qk_head(1, h, ETs[1])
                    if h >= 1:
                        emit_pv_head(1, h - 1)
                    if h % 3 == 2:
                        emit_y(0, h // 3)
                emit_pv_head(1, NH - 1)
                for ni in range(NQC):
                    emit_y(1, ni)
                ET1_holder.append(ET1)

            if DBG_DUMP:
                nc.sync.dma_start(out=dbg_qT, in_=qT[1])
                nc.sync.dma_start(out=dbg_kT, in_=kT[1])
                et1v = ET1_holder[0].rearrange("p (c n) -> p c n", n=N)
                nc.sync.dma_start(out=dbg_ET, in_=et1v)
                nc.sync.dma_start(out=dbg_vsb, in_=vsb[1])
                nc.sync.dma_start(out=dbg_OT, in_=OT[1])

    nc.compile()
    return nc


def _pack_inputs(x, sp, ed, Wq, Wk, Wv, bv, Wo, bo):
    """Host-side layout/dtype marshalling (pure data movement + weight
    preprocessing; all activation arithmetic happens on-device)."""
    x = np.asarray(x, np.float32)
    sp = np.asarray(sp, np.float32)
    ed = np.asarray(ed, np.float32)
    Wq = np.asarray(Wq, np.float32)
    Wk = np.asarray(Wk, np.float32)
    Wv = np.asarray(Wv, np.float32)
    Wo = np.asarray(Wo, np.float32)
    bv = np.asarray(bv, np.float32)
    bo = np.asarray(bo, np.float32)

    xT = np.ascontiguousarray(x.transpose(0, 2, 1))          # [B, H, N]
    xb = np.ascontiguousarray(
        xT.reshape(B, KC, P, N).transpose(0, 2, 1, 3)).astype(BFNP)
    spT = np.ascontiguousarray(
        sp.transpose(0, 2, 1).reshape(B, NQC, P, N).transpose(0, 2, 1, 3))
    edT = np.ascontiguousarray(
        ed.transpose(0, 2, 1).reshape(B, NQC, P, N).transpose(0, 2, 1, 3))

    def packb(W, mul=1.0):
        WT = np.ascontiguousarray(W.T * mul)
        return np.ascontiguousarray(
            WT.reshape(KC, P, H).transpose(1, 0, 2)).astype(BFNP)

    shared = {
        "wqT": packb(Wq, SCALE), "wkT": packb(Wk),
        "wvT": packb(Wv), "woT": packb(Wo),
        "bo2": (bo + bv @ Wo.T).astype(BFNP),
    }
    return xb, spT, edT, shared


def kernel(x, spatial_encoding, edge_encoding, Wq, Wk, Wv, bv, Wo, bo):
    global _COMPILED
    from concourse.bass_utils import run_bass_kernel_spmd

    if _COMPILED is None:
        _COMPILED = _build()
    nc = _COMPILED

    xb, spT, edT, shared = _pack_inputs(
        x, spatial_encoding, edge_encoding, Wq, Wk, Wv, bv, Wo, bo)

    in_maps = []
    for c in range(NCORES):
        sl = slice(c * PB, (c + 1) * PB)
        in_maps.append({"xb": xb[sl],
                        "spT": spT[sl], "edT": edT[sl], **shared})

    res = run_bass_kernel_spmd(nc, in_maps, list(range(NCORES)))
    return np.concatenate([res.results[c]["y"] for c in range(NCORES)], axis=0)


# revision 19
# speedup vs baseline: 1.3664x; 1.0799x over previous
"""Graphormer multi-head attention on 8 Trainium2 NeuronCores.

Problem (hardcoded): B=16, N=512, HIDDEN=768, 12 heads x 64.
  q = x @ Wq.T ; k = x @ Wk.T ; v = x @ Wv.T + bv
  scores = (q.k / sqrt(768)) + (spatial + edge)[:, None]
  out = softmax(scores) @ v ; y = out @ Wo.T + bo
Sharding: data-parallel over batch, 2 batches per core on 8 cores.

Per-core kernel strategy:
  - All layout work happens on the HOST: x/spatial/edge are pre-transposed
    and packed; weights are pre-transposed and cast to bf16.  bv is
    folded into bo' = bo + Wo@bv on the host (valid because softmax rows
    sum to 1).  No PE transposes.
  - All matmuls bf16 with fp32 PSUM accumulation; the 1/sqrt(768)
    softmax scale is folded into Wq on the host.
  - Attention in the S^T layout: S.T[nk, nq] = kT.T @ qT per head.
    exp(S^T * s) on ScalarE -> bf16, then one DVE multiply with
    E^T = exp((spatial+edge)^T) shared across heads.
  - PV per head: lhsT = [v_head | ones] (ones live in dedicated slots of
    the v tile), so one matmul chain yields both O^T (64 rows) and the
    softmax denominator replicated across the other 64 rows -- no
    separate row-sum matmuls.  A tiny stride-0-partition DMA broadcasts
    the denominator row into SBUF on the partitions where O^T lives,
    keeping the reciprocal+normalize DVE ops partition-aligned.
  - y = O @ Wo.T + bo' with lhsT = O^T; bo' enters as a K=1 matmul and
    the result is DMA'd to DRAM straight out of PSUM.
  - Emission interleaves batch b+1's projections into batch b's
    attention loop so the PE never starves.
"""

import numpy as np
import ml_dtypes

B, N, H = 16, 512, 768
NH, HD = 12, 64
NCORES = 8
PB = B // NCORES  # batches per core
P = 128
KC = H // P   # 6 hidden chunks of 128
NQC = N // P  # 4 sequence chunks of 128
SCALE = float(H) ** -0.5        # folded into Wq on the host

BFNP = ml_dtypes.bfloat16

_COMPILED = None

# debug toggles (set before _build for HW-vs-sim bisection)
DBG_NO_PBCAST = False    # replace partition_broadcast with memset(1.0)
DBG_NO_ACCUM = False     # replace DMA-accum bias sum with DVE add
DBG_NO_GMEMSET = False   # vsb ones via DVE memset instead of gpsimd
DBG_SEQUENTIAL = False   # no cross-batch interleaving in emission order
DBG_DUMP = False         # DMA b1 intermediates to DRAM debug outputs


def _build():
    import concourse.bacc as bacc
    import concourse.tile as tile
    import concourse.mybir as mybir

    f32 = mybir.dt.float32
    bf16 = mybir.dt.bfloat16
    Exp = mybir.ActivationFunctionType.Exp
    ADD = mybir.AluOpType.add

    nc = bacc.Bacc("TRN2", target_bir_lowering=False, debug=False,
                   enable_asserts=False, num_devices=NCORES)

    xb_d = nc.dram_tensor("xb", [PB, P, KC, N], bf16, kind="ExternalInput").ap()
    sp_d = nc.dram_tensor("spT", [PB, P, NQC, N], f32, kind="ExternalInput").ap()
    ed_d = nc.dram_tensor("edT", [PB, P, NQC, N], f32, kind="ExternalInput").ap()
    wq_d = nc.dram_tensor("wqT", [P, KC, H], bf16, kind="ExternalInput").ap()
    wk_d = nc.dram_tensor("wkT", [P, KC, H], bf16, kind="ExternalInput").ap()
    wv_d = nc.dram_tensor("wvT", [P, KC, H], bf16, kind="ExternalInput").ap()
    wo_d = nc.dram_tensor("woT", [P, KC, H], bf16, kind="ExternalInput").ap()
    bo_d = nc.dram_tensor("bo2", [H], bf16, kind="ExternalInput").ap()
    y_d = nc.dram_tensor("y", [PB, N, H], f32, kind="ExternalOutput").ap()
    if DBG_DUMP:
        dbg_qT = nc.dram_tensor("dbg_qT", [P, KC, N], bf16, kind="ExternalOutput").ap()
        dbg_kT = nc.dram_tensor("dbg_kT", [P, KC, N], bf16, kind="ExternalOutput").ap()
        dbg_ET = nc.dram_tensor("dbg_ET", [P, NQC, N], bf16, kind="ExternalOutput").ap()
        dbg_vsb = nc.dram_tensor("dbg_vsb", [P, NQC, NH, P], bf16, kind="ExternalOutput").ap()
        dbg_OT = nc.dram_tensor("dbg_OT", [P, KC, N], bf16, kind="ExternalOutput").ap()

    with tile.TileContext(nc) as tc:
        with (
            tc.tile_pool(name="consts", bufs=1) as consts,
            tc.tile_pool(name="weights", bufs=1) as weights,
            tc.tile_pool(name="io", bufs=2) as io,
            tc.tile_pool(name="biasp", bufs=2) as biasp,
            tc.tile_pool(name="qk", bufs=2) as qkp,
            tc.tile_pool(name="vp", bufs=1) as vp,
            tc.tile_pool(name="soft", bufs=3) as soft,
            tc.tile_pool(name="zp", bufs=2) as zp,
            tc.tile_pool(name="op", bufs=2) as op_,
            tc.tile_pool(name="psum", bufs=2, space="PSUM") as psum,
        ):
            ones_r1 = consts.tile([1, P], bf16)
            nc.vector.memset(ones_r1, 1.0)

            # v tiles: each head owns a contiguous 128-wide slot holding
            # [v_h | ones]; the ones columns make the PV matmul emit the
            # softmax denominator, replicated across PSUM partitions
            # 64:128, for free.
            vsb = [vp.tile([P, NQC, NH, P], bf16, name=f"vsb{i}")
                   for i in range(PB)]
            for t in vsb:
                nc.gpsimd.memset(t[:, :, :, HD:P], 1.0)

            # DMA issue order is the PE warm-up critical path: the q/k
            # projections need wqT/wkT + xb(0) first; the attention needs the
            # bias exp soon after; everything else can trail.
            wqT = weights.tile([P, KC, H], bf16)
            nc.sync.dma_start(out=wqT, in_=wq_d)
            xbt = []
            xb_t0 = io.tile([P, KC, N], bf16, tag="xb", name="xb0")
            nc.sync.dma_start(out=xb_t0, in_=xb_d[0])
            xbt.append(xb_t0)
            wkT = weights.tile([P, KC, H], bf16)
            nc.sync.dma_start(out=wkT, in_=wk_d)

            def emit_bias(b):
                bsum = biasp.tile([P, NQC, N], f32, tag="bsum", name="bsum")
                if DBG_NO_ACCUM:
                    esb = biasp.tile([P, NQC, N], f32, tag="esb", name="esb")
                    nc.sync.dma_start(out=bsum, in_=sp_d[b])
                    nc.sync.dma_start(out=esb, in_=ed_d[b])
                    nc.vector.tensor_add(bsum, bsum, esb)
                else:
                    nc.gpsimd.dma_start(out=bsum, in_=sp_d[b])
                    nc.gpsimd.dma_start(out=bsum, in_=ed_d[b], accum_op=ADD)
                ET = biasp.tile([P, NQC, N], bf16, tag="ET", name="ET")
                nc.scalar.activation(ET, bsum, Exp)
                return ET.rearrange("p c n -> p (c n)")

            wvT = weights.tile([P, KC, H], bf16)
            nc.sync.dma_start(out=wvT, in_=wv_d)
            woT = weights.tile([P, KC, H], bf16)
            nc.sync.dma_start(out=woT, in_=wo_d)
            xb_t1 = io.tile([P, KC, N], bf16, tag="xb", name="xb1")
            nc.sync.dma_start(out=xb_t1, in_=xb_d[1])
            xbt.append(xb_t1)
            bo2_sb = consts.tile([1, H], bf16)
            nc.sync.dma_start(out=bo2_sb, in_=bo_d[None, :])

            qT, kT, OT = {}, {}, {}

            def emit_projqk(b, mi):
                for wt, dst in ((wqT, qT), (wkT, kT)):
                    pp = psum.tile([P, N], f32, tag="pp", name="pp")
                    for kc in range(KC):
                        nc.tensor.matmul(
                            pp, wt[:, kc, mi * P:(mi + 1) * P],
                            xbt[b][:, kc, :],
                            start=(kc == 0), stop=(kc == KC - 1))
                    nc.scalar.copy(dst[b][:, mi, :], pp)

            def emit_projv(b, ni):
                for half in range(2):
                    cols = slice(0, N) if half == 0 else slice(N, H)
                    width = cols.stop - cols.start
                    nh = width // HD       # heads covered by this psum tile
                    h0 = half * 8          # first head
                    pv = psum.tile([P, N], f32, tag="pp", name="pv")
                    for kc in range(KC):
                        nc.tensor.matmul(
                            pv[:, 0:width],
                            xbt[b][:, kc, ni * P:(ni + 1) * P],
                            wvT[:, kc, cols],
                            start=(kc == 0), stop=(kc == KC - 1))
                    pvh = pv[:, 0:width].rearrange("p (h d) -> p h d", d=HD)
                    nc.vector.tensor_copy(
                        vsb[b][:, ni, h0:h0 + nh, 0:HD], pvh)

            es_t = {}

            def emit_qk_head(b, h, ETflat):
                mi, r0 = h // 2, (h % 2) * HD
                rs = slice(r0, r0 + HD)
                es = soft.tile([P, 2 * N * 2], bf16, tag="es", name="es")
                for half in range(2):
                    pq = psum.tile([P, 2 * N], f32, tag="pqk", name="pq")
                    for j in range(2):
                        ki = 2 * half + j
                        nc.tensor.matmul(
                            pq[:, j * N:(j + 1) * N],
                            kT[b][rs, mi, ki * P:(ki + 1) * P],
                            qT[b][rs, mi, :],
                            start=True, stop=True)
                    nc.scalar.activation(
                        es[:, half * 2 * N:(half + 1) * 2 * N], pq, Exp)
                nc.vector.tensor_mul(es, es, ETflat)
                es_t[(b, h)] = es

            def emit_pv_head(b, h):
                es = es_t.pop((b, h))
                po = psum.tile([P, N], f32, tag="po", name="po")
                for ki in range(NQC):
                    # [v|ones] -> O rows 0:64, Z (replicated) rows 64:128
                    nc.tensor.matmul(po, vsb[b][:, ki, h, :],
                                     es[:, ki * N:(ki + 1) * N],
                                     start=(ki == 0), stop=(ki == NQC - 1))
                # move Z onto partitions 0:64 (cross-offset copy is the only
                # partition-crossing engine op that works on HW), reciprocal
                # at offset 0, then scale O into its OT slot (cross-offset
                # output is fine when both inputs share base partition 0).
                rr = zp.tile([P, N], f32, tag="rr", name="rr")
                nc.scalar.copy(rr[0:HD, :], po[HD:P, :])
                nc.vector.reciprocal_approx_fast(rr[0:HD, :], rr[0:HD, :])
                orow = slice((h % 2) * HD, (h % 2) * HD + HD)
                nc.vector.tensor_mul(OT[b][orow, h // 2, :], po[0:HD, :],
                                     rr[0:HD, :])

            def emit_y(b, ni):
                ysb = op_.tile([P, H], f32, tag="ysb", name="ysb", bufs=3)
                for half in range(2):
                    cols = slice(0, N) if half == 0 else slice(N, H)
                    width = cols.stop - cols.start
                    py = psum.tile([P, N], f32, tag="pp", name="py")
                    for jc in range(KC):
                        nc.tensor.matmul(
                            py[:, 0:width],
                            OT[b][:, jc, ni * P:(ni + 1) * P],
                            woT[:, jc, cols],
                            start=(jc == 0), stop=False)
                    nc.tensor.matmul(py[:, 0:width], ones_r1,
                                     bo2_sb[:, cols], start=False, stop=True)
                    if half == 0:
                        nc.scalar.copy(ysb[:, cols], py[:, 0:width])
                    else:
                        nc.vector.tensor_copy(ysb[:, cols], py[:, 0:width])
                nc.sync.dma_start(
                    out=y_d[b, ni * P:(ni + 1) * P, :], in_=ysb)

            # ---------------- emission schedule ----------------
            ET0 = emit_bias(0)
            for b in range(PB):
                qT[b] = qkp.tile([P, KC, N], bf16, tag="qT", name="qT")
                kT[b] = qkp.tile([P, KC, N], bf16, tag="kT", name="kT")
                OT[b] = op_.tile([P, KC, N], bf16, tag="OT", name="OT")

            ET1_holder = []
            if DBG_SEQUENTIAL:
                ET1 = emit_bias(1)
                ET1_holder.append(ET1)
                for b in range(PB):
                    for mi in range(KC):
                        emit_projqk(b, mi)
                    for ni in range(NQC):
                        emit_projv(b, ni)
                    for h in range(NH):
                        emit_qk_head(b, h, [ET0, ET1][b])
                        emit_pv_head(b, h)
                    for ni in range(NQC):
                        emit_y(b, ni)
            else:
                for mi in range(KC):
                    emit_projqk(0, mi)
                for ni in range(NQC):
                    emit_projv(0, ni)
                ET1 = emit_bias(1)
                ETs = [ET0, ET1]

                # attention(0) interleaved with projections(1)
                b1_chunks = ([("qk", mi) for mi in range(KC)]
                             + [("v", ni) for ni in range(NQC)])
                ci = 0
                for h in range(NH):
                    emit_qk_head(0, h, ETs[0])
                    if h >= 1:
                        emit_pv_head(0, h - 1)
                    if ci < len(b1_chunks):
                        kind, idx = b1_chunks[ci]
                        ci += 1
                        (emit_projqk if kind == "qk" else emit_projv)(1, idx)
                while ci < len(b1_chunks):
                    kind, idx = b1_chunks[ci]
                    ci += 1
                    (emit_projqk if kind == "qk" else emit_projv)(1, idx)
                emit_pv_head(0, NH - 1)

                # attention(1) interleaved with output proj(0)
                for h in range(NH):
                    emit_qk_head(1, h, ETs[1])
                    if h >= 1:
                        emit_pv_head(1, h - 1)
                    if h % 3 == 2:
                        emit_y(0, h // 3)
                emit_pv_head(1, NH - 1)
                for ni in range(NQC):
                    emit_y(1, ni)
                ET1_holder.append(ET1)

            if DBG_DUMP:
                nc.sync.dma_start(out=dbg_qT, in_=qT[1])
                nc.sync.dma_start(out=dbg_kT, in_=kT[1])
                et1v = ET1_holder[0].rearrange("p (c n) -> p c n", n=N)
                nc.sync.dma_start(out=dbg_ET, in_=et1v)
                nc.sync.dma_start(out=dbg_vsb, in_=vsb[1])
                nc.sync.dma_start(out=dbg_OT, in_=OT[1])

    nc.compile()
    return nc


def _pack_inputs(x, sp, ed, Wq, Wk, Wv, bv, Wo, bo):
    """Host-side layout/dtype marshalling (pure data movement + weight
    preprocessing; all activation arithmetic happens on-device)."""
    x = np.asarray(x, np.float32)
    sp = np.asarray(sp, np.float32)
    ed = np.asarray(ed, np.float32)
    Wq = np.asarray(Wq, np.float32)
    Wk = np.asarray(Wk, np.float32)
    Wv = np.asarray(Wv, np.float32)
    Wo = np.asarray(Wo, np.float32)
    bv = np.asarray(bv, np.float32)
    bo = np.asarray(bo, np.float32)

    xT = np.ascontiguousarray(x.transpose(0, 2, 1))          # [B, H, N]
    xb = np.ascontiguousarray(
        xT.reshape(B, KC, P, N).transpose(0, 2, 1, 3)).astype(BFNP)
    spT = np.ascontiguousarray(
        sp.transpose(0, 2, 1).reshape(B, NQC, P, N).transpose(0, 2, 1, 3))
    edT = np.ascontiguousarray(
        ed.transpose(0, 2, 1).reshape(B, NQC, P, N).transpose(0, 2, 1, 3))

    def packb(W, mul=1.0):
        WT = np.ascontiguousarray(W.T * mul)
        return np.ascontiguousarray(
            WT.reshape(KC, P, H).transpose(1, 0, 2)).astype(BFNP)

    shared = {
        "wqT": packb(Wq, SCALE), "wkT": packb(Wk),
        "wvT": packb(Wv), "woT": packb(Wo),
        "bo2": (bo + bv @ Wo.T).astype(BFNP),
    }
    return xb, spT, edT, shared


def kernel(x, spatial_encoding, edge_encoding, Wq, Wk, Wv, bv, Wo, bo):
    global _COMPILED
    from concourse.bass_utils import run_bass_kernel_spmd

    if _COMPILED is None:
        _COMPILED = _build()
    nc = _COMPILED

    xb, spT, edT, shared = _pack_inputs(
        x, spatial_encoding, edge_encoding, Wq, Wk, Wv, bv, Wo, bo)

    in_maps = []
    for c in range(NCORES):
        sl = slice(c * PB, (c + 1) * PB)
        in_maps.append({"xb": xb[sl],
                        "spT": spT[sl], "edT": edT[sl], **shared})

    res = run_bass_kernel_spmd(nc, in_maps, list(range(NCORES)))
    return np.concatenate([res.results[c]["y"] for c in range(NCORES)], axis=0)
